# revision 2
# baseline (speedup 1.0000x reference)
"""Dot-product attention v2 on 8 Trainium2 NeuronCores.

Full inputs [B=4, H=16, S=1024, D=64] fp32. 64 heads sharded 8-per-core,
processed in head PAIRS (A = rows 0-63 of the PE array, B = rows 64-127).

Differences vs v1 (136us):
 - exp stage PSUM is double-buffered ([128,1536] x 2 = 6 banks), so ScalarE
   runs exp back-to-back instead of serializing with MM1 stage fills.
 - MM2 is column-packed: head A -> PE cols 0-63 (out partitions 0-63),
   head B -> cols 64-127, concurrent, one PSUM bank per c-half.
 - softmax sums come from 4-way col-tiled [K=128, M=32] ones-matmuls
   (32-replicated rows -> whole sums bank live, reciprocal needs no masking,
   no DVE row copies).
 - normalize = one [128,512] scalar_tensor_tensor per c-half reading both
   PSUM operands (bcast matmul result and unnormalized out) directly.
 - PE emission order keeps bcast matmuls (which wait on the reciprocal)
   after the phase's MM1 fills so the in-order PE queue never starves
   ScalarE.

Pipeline phase p: MM1+exp of pair p, MM2 + sums + recip + bcast + stt of
pair p-1.

Toolchain notes (walrus 2026-05-04 + bass_rust skew), carried from v1:
 - walrus accepts at most ONE sync-wait per instruction; a JSON pass over
   the BIR hoists extra waits onto NoOps. TileContext tail drain patched
   the same way.
 - fp32r matmul operands must be produced as float32r, not bitcast.
"""

import json
from contextlib import ExitStack

import numpy as np

import concourse.bass as bass
import concourse.bass2jax as bass2jax
import concourse.mybir as mybir
import concourse.tile as tile
from concourse import bass_utils
from concourse.vector_clock import ScopedClock

F32 = mybir.dt.float32
F32R = mybir.dt.float32r
BF16 = mybir.dt.bfloat16

N_CORES = 8
HEADS_PER_CORE = 8
NPAIR = HEADS_PER_CORE // 2
S = 1024
D = 64
KT = S // 128  # 8 k-tiles per head
PAIR_COLS = 2 * S * KT  # 16384 exp columns per pair
CHUNK = 1536  # exp chunk width (3 PSUM banks); last chunk is 1024
CHUNKS = [(i * CHUNK, min(CHUNK, PAIR_COLS - i * CHUNK))
          for i in range((PAIR_COLS + CHUNK - 1) // CHUNK)]
# pair 0 starts with a narrow chunk so the first exp only needs the first
# DMA piece and one cold matmul
CHUNKS_P0 = [(0, 512)] + [(512 + i * CHUNK, CHUNK) for i in range(10)] + [
    (512 + 10 * CHUNK, 512)]

_DRAIN_MAX_WAITS = 1


def _split_drain_and_barrier(self, tick_clock, wait_clock):
    nc = self.nc
    drain_inst = nc.sync.drain()
    wait_clock.add_sem_waits(
        drain_inst.ins, ScopedClock({None: tick_clock.global_clock})
    )
    si = drain_inst.ins.sync_info
    if si is not None and si.on_wait and len(si.on_wait) > _DRAIN_MAX_WAITS:
        waits = list(si.on_wait)
        updates = list(si.on_update or [])
        drain_inst.ins.sync_info = mybir.SyncInfo(
            on_wait=waits[:_DRAIN_MAX_WAITS], on_update=[]
        )
        rest = waits[_DRAIN_MAX_WAITS:]
        for i in range(0, len(rest), _DRAIN_MAX_WAITS):
            extra = nc.sync.drain()
            extra.ins.sync_info = mybir.SyncInfo(
                on_wait=rest[i : i + _DRAIN_MAX_WAITS],
                on_update=updates if i + _DRAIN_MAX_WAITS >= len(rest) else [],
            )
    nc.all_engine_barrier()
    assert self.sems is not None
    popped = nc._tile_sem_poison_stack.pop()
    assert popped is self._sem_poison
    nc.clear_and_free_semaphores(list(self.sems.allocated().values()))
    nc.all_engine_barrier()


def _split_waits_in_bir(bir_json: bytes) -> bytes:
    """Hoist extra sync-waits onto NoOps inserted immediately before the
    owning instruction (same engine, in-order => semantics unchanged)."""
    j = json.loads(bir_json)
    n = 0
    for f in j["functions"]:
        for b in f["blocks"]:
            out = []
            for inst in b["instructions"]:
                si = inst.get("sync_info")
                waits = (si or {}).get("on_wait") or []
                if len(waits) > 1:
                    for w in waits[:-1]:
                        out.append(
                            {
                                "debug": inst.get("debug", 0),
                                "engine": inst["engine"],
                                "ins": [],
                                "outs": [],
                                "name": f"{inst['name']}-wsplit{n}",
                                "opcode": "NoOp",
                                "sync_info": {"on_update": [], "on_wait": [w]},
                            }
                        )
                        n += 1
                    si["on_wait"] = [waits[-1]]
                out.append(inst)
            b["instructions"] = out
    return json.dumps(j).encode()


_orig_compile_bir_kernel = bass_utils.compile_bir_kernel


def _compile_bir_kernel_splitting(bir_json, tmpdir, neff_name="file.neff"):
    return _orig_compile_bir_kernel(_split_waits_in_bir(bir_json), tmpdir, neff_name)


ENABLE_LDW_OPT = False
_orig_run_command = bass_utils.run_command


def _run_command_ldw(argv, **kwargs):
    if ENABLE_LDW_OPT:
        argv = [
            a.replace("--enable-ldw-opt=false", "--enable-ldw-opt=true") for a in argv
        ]
    return _orig_run_command(argv, **kwargs)


def _install_patches():
    if not getattr(tile.TileContext, "_drain_split_installed", False):
        tile.TileContext._drain_and_barrier = _split_drain_and_barrier
        tile.TileContext._drain_split_installed = True
    if bass_utils.compile_bir_kernel is not _compile_bir_kernel_splitting:
        bass_utils.compile_bir_kernel = _compile_bir_kernel_splitting
        bass2jax.compile_bir_kernel = _compile_bir_kernel_splitting
        bass_utils.run_command = _run_command_ldw


def build_nc(scale: float) -> bass.Bass:
    _install_patches()
    nc = bass.Bass(
        trn_type="TRN2", target_bir_lowering=False, debug=False, num_devices=N_CORES
    )
    # kq[pair, 0:64, 0:1024] = Q^T head 2p ; [0:64, 1024:] = K^T head 2p
    # kq[pair, 64:128, ...]  = same for head 2p+1    (d-major, fp32r)
    kq = nc.dram_tensor(
        "kq", [NPAIR, 128, 2 * S], F32R, kind="ExternalInput"
    ).ap()
    # vt[h, p, t, j]: V[h, 128*t + p, j]  (bf16, no ones column)
    vt = nc.dram_tensor(
        "vt", [HEADS_PER_CORE, 128, KT, D], BF16, kind="ExternalInput"
    ).ap()
    ones32 = nc.dram_tensor("ones32", [128, 32], BF16, kind="ExternalInput").ap()
    # sels2[k, qc, m] = 1.0 where k == 32*(2*qc + m//64): one matmul
    # broadcasts sums row 32*2qc across out partitions 0-63 (head A) and row
    # 32*(2qc+1) across 64-127 (head B).
    sels_d = nc.dram_tensor("sels2", [128, 2, 128], F32R, kind="ExternalInput").ap()
    outT = nc.dram_tensor(
        "outT", [HEADS_PER_CORE, D, S], F32, kind="ExternalOutput"
    ).ap()

    with tile.TileContext(nc) as tc, ExitStack() as ctx:
        sb = ctx.enter_context(tc.tile_pool(name="sb", bufs=2))
        singles = ctx.enter_context(tc.tile_pool(name="singles", bufs=1))
        # PSUM: 2 x [128,1536] stages (6 banks) + o bank + sums/bc bank
        ps_stage = ctx.enter_context(
            tc.tile_pool(name="ps_stage", bufs=2, space="PSUM"))
        ps_o = ctx.enter_context(tc.tile_pool(name="ps_o", bufs=1, space="PSUM"))
        ps_y = ctx.enter_context(tc.tile_pool(name="ps_y", bufs=1, space="PSUM"))

        state = {}  # pair -> dict(kq_s, va, vb, e, o2)

        def prefetch(p, split_first=False):
            kq_s = sb.tile([128, 2 * S], F32R, tag="kq")
            if split_first:
                # head A's q c0 + k-tile 0 land first so MM1 span 0 (rows
                # 0-63 only) starts as early as possible
                nc.sync.dma_start(kq_s[0:64, :512], kq[p][0:64, :512])
                nc.sync.dma_start(
                    kq_s[0:64, S : S + 128], kq[p][0:64, S : S + 128])
                nc.sync.dma_start(kq_s[64:128, :512], kq[p][64:128, :512])
                nc.sync.dma_start(
                    kq_s[64:128, S : S + 128], kq[p][64:128, S : S + 128])
                nc.sync.dma_start(kq_s[:, S + 128 : S + 256],
                                  kq[p][:, S + 128 : S + 256])
                nc.sync.dma_start(kq_s[:, 512:S], kq[p][:, 512:S])
                nc.sync.dma_start(kq_s[:, S + 256 :], kq[p][:, S + 256 :])
            else:
                nc.sync.dma_start(kq_s[:, :S], kq[p][:, :S])
                nc.sync.dma_start(kq_s[:, S:], kq[p][:, S:])
            va = sb.tile([128, KT, D], BF16, tag="va")
            nc.gpsimd.dma_start(va, vt[2 * p])
            vb = sb.tile([128, KT, D], BF16, tag="vb")
            nc.gpsimd.dma_start(vb, vt[2 * p + 1])
            e = sb.tile([128, PAIR_COLS], BF16, tag="e")
            state[p] = {"kq": kq_s, "va": va, "vb": vb, "e": e}

        # e column layout: g = ki*2048 + (2*qc + half)*512 + q  — consecutive
        # 512-spans alternate head halves so every MM1 LDWEIGHTS overlaps the
        # opposite row-group's in-flight matmul stream.
        def emit_mm1_chunk(p, chunks, ci):
            """Fill stage chunk ci ([128, w] scores^T) and exp it to e."""
            off, w = chunks[ci]
            kq_s = state[p]["kq"]
            stage = ps_stage.tile([128, CHUNK], F32, tag="stage")
            for so in range(0, w, 512):
                sp = (off + so) // 512
                ki, qc, half = sp // 4, (sp // 2) % 2, sp % 2
                base = 64 * half
                nc.tensor.matmul(
                    stage[:, so : so + 512],
                    kq_s[base : base + 64, S + ki * 128 : S + (ki + 1) * 128],
                    kq_s[base : base + 64, qc * 512 : (qc + 1) * 512],
                    start=True,
                    stop=True,
                )
            nc.scalar.activation(
                out=state[p]["e"][:, off : off + w],
                in_=stage[:, :w],
                func=mybir.ActivationFunctionType.Exp,
                scale=scale,
            )

        # sums rows 32g, g = 2*qc + half = e col block index
        def emit_sums_round(p, ki, su_ps):
            e = state[p]["e"]
            for g in range(4):
                nc.tensor.matmul(
                    su_ps[32 * g : 32 * (g + 1), :],
                    ones_s,
                    e[:, ki * 2048 + g * 512 : ki * 2048 + (g + 1) * 512],
                    start=(ki == 0),
                    stop=(ki == KT - 1),
                    tile_position=(0, 32 * g),
                )

        def emit_mm2_step(p, qc, ki, o_ps):
            va, vb, e = state[p]["va"], state[p]["vb"], state[p]["e"]
            for half, v_s in ((0, va), (1, vb)):
                blk = 2 * qc + half
                nc.tensor.matmul(
                    o_ps[64 * half : 64 * half + 64, :],
                    v_s[:, ki, :],
                    e[:, ki * 2048 + blk * 512 : ki * 2048 + (blk + 1) * 512],
                    start=(ki == 0),
                    stop=(ki == KT - 1),
                    tile_position=(0, 64 * half),
                )

        def emit_recip(su_ps):
            recip = sb.tile([128, 512], F32R, tag="recip")
            with nc.allow_low_precision(reason="fp32r recip for bcast matmul"):
                nc.vector.reciprocal(out=recip, in_=su_ps)
            return recip

        def emit_bc(qc, recip, bc_ps):
            # single K=64 fp32r matmul: out rows 0-63 get 1/sums_A(qc), rows
            # 64-127 get 1/sums_B(qc) (selector one-hot rows are all in hr)
            hr = slice(0, 64) if qc == 0 else slice(64, 128)
            nc.tensor.matmul(
                bc_ps,
                sels_s[hr, qc, :],
                recip[hr, :],
                start=True,
                stop=True,
            )

        def emit_ou_copy(o_ps):
            # DVE may read only one PSUM operand per instruction: stash the
            # unnormalized out in SBUF so the stt can read bc from PSUM.
            ou = sb.tile([128, 512], F32, tag="ou")
            nc.vector.tensor_copy(ou, o_ps)
            return ou

        def emit_stt(p, qc, ou, bc_ps):
            o2 = state[p]["o2"]
            nc.vector.scalar_tensor_tensor(
                out=o2[:, qc * 512 : (qc + 1) * 512],
                in0=bc_ps,
                scalar=1.0,
                op0=mybir.AluOpType.mult,
                in1=ou,
                op1=mybir.AluOpType.mult,
            )

        prefetch(0, split_first=True)
        sels_s = singles.tile([128, 2, 128], F32R, tag="sels")
        nc.sync.dma_start(sels_s, sels_d)
        ones_s = singles.tile([128, 32], BF16, tag="ones")
        nc.gpsimd.dma_start(ones_s, ones32)

        # Phase p (slots ci, ~1.54us each = one exp chunk):
        #   ci 0-7:  MM1(p,ci) + MM2(p-1,qc0,ki=ci) + sums(p-1,ki=ci)
        #   ci 7:    ou0 copy (DVE) after MM2 qc0 completes
        #   ci 8:    recip (DVE, ~3.4us, hidden) ; MM2 qc1 starts (2/slot)
        #   post:    MM2 qc1 tail, bc+stt both halves, out DMAs
        # PE total/phase ~15us < ScalarE 16.9us, and no PE instruction early
        # in the queue ever waits on the reciprocal.
        for p in range(NPAIR + 1):
            cur = p < NPAIR  # this phase has MM1/exp work
            prv = p >= 1  # this phase has MM2/normalize work for p-1
            chunks = CHUNKS_P0 if p == 0 else CHUNKS
            if cur:
                o2_t = sb.tile([128, 2 * 512], F32, tag="o2")
                state[p]["o2"] = o2_t
            if prv:
                su_ps = ps_y.tile([128, 512], F32, tag="y")
                o_c0 = ps_o.tile([128, 512], F32, tag="o")
            if cur:
                for ci in range(len(chunks)):
                    emit_mm1_chunk(p, chunks, ci)
                    if ci == 4 and p + 1 < NPAIR:
                        prefetch(p + 1)
                    if prv:
                        if ci < 4:
                            emit_sums_round(p - 1, 2 * ci, su_ps)
                            emit_sums_round(p - 1, 2 * ci + 1, su_ps)
                        if ci == 4:
                            recip = emit_recip(su_ps)
                        if 4 <= ci < 8:
                            emit_mm2_step(p - 1, 0, 2 * (ci - 4), o_c0)
                            emit_mm2_step(p - 1, 0, 2 * (ci - 4) + 1, o_c0)
                        if ci == 7:
                            ou0 = emit_ou_copy(o_c0)
                        if ci == 8:
                            o_c1 = ps_o.tile([128, 512], F32, tag="o")
                        if 8 <= ci <= 10:
                            emit_mm2_step(p - 1, 1, 2 * (ci - 8), o_c1)
                            emit_mm2_step(p - 1, 1, 2 * (ci - 8) + 1, o_c1)
            elif prv:
                # tail: interleave sums with MM2 qc0; queue qc1 on PE before
                # the reciprocal so it overlaps the DVE divide
                for ki in range(8):
                    emit_sums_round(p - 1, ki, su_ps)
                    emit_mm2_step(p - 1, 0, ki, o_c0)
                ou0 = emit_ou_copy(o_c0)
                o_c1 = ps_o.tile([128, 512], F32, tag="o")
                for ki in range(6):
                    emit_mm2_step(p - 1, 1, ki, o_c1)
                recip = emit_recip(su_ps)
            if prv:
                emit_mm2_step(p - 1, 1, 6, o_c1)
                emit_mm2_step(p - 1, 1, 7, o_c1)
                bc_c0 = ps_y.tile([128, 512], F32, tag="y")
                emit_bc(0, recip, bc_c0)
                emit_stt(p - 1, 0, ou0, bc_c0)
                ou1 = emit_ou_copy(o_c1)
                bc_c1 = ps_y.tile([128, 512], F32, tag="y")
                emit_bc(1, recip, bc_c1)
                emit_stt(p - 1, 1, ou1, bc_c1)
                o2 = state[p - 1]["o2"]
                nc.sync.dma_start(outT[2 * (p - 1)], o2[0:64, :])
                nc.sync.dma_start(outT[2 * (p - 1) + 1], o2[64:128, :])

    return nc


def _shard_inputs(queries, keys, values):
    """Full [4,16,1024,64] fp32 -> per-core kq (fp32r) / vt (bf16)."""
    import ml_dtypes

    q = np.ascontiguousarray(queries, dtype=np.float32).reshape(64, S, D)
    k = np.ascontiguousarray(keys, dtype=np.float32).reshape(64, S, D)
    v = np.ascontiguousarray(values, dtype=np.float32).reshape(64, S, D)

    qT = q.transpose(0, 2, 1)  # [64, D, S]
    kT = k.transpose(0, 2, 1)

    kq = np.empty((64 // 2, 128, 2 * S), np.float32)
    kq[:, 0:64, 0:S] = qT[0::2]
    kq[:, 0:64, S:] = kT[0::2]
    kq[:, 64:128, 0:S] = qT[1::2]
    kq[:, 64:128, S:] = kT[1::2]

    vt = np.ascontiguousarray(
        v.reshape(64, KT, 128, D).transpose(0, 2, 1, 3)
    ).astype(ml_dtypes.bfloat16)

    ones32 = np.ones((128, 32), ml_dtypes.bfloat16)

    sels = np.zeros((128, 2, 128), np.float32)
    for qc in range(2):
        for m in range(128):
            sels[32 * (2 * qc + m // 64), qc, m] = 1.0

    in_maps = []
    for c in range(N_CORES):
        in_maps.append(
            {
                "kq": np.ascontiguousarray(kq[c * 4 : (c + 1) * 4]),
                "vt": np.ascontiguousarray(vt[c * 8 : (c + 1) * 8]),
                "ones32": np.asarray(ones32),
                "sels2": sels,
            }
        )
    return in_maps


_CACHE = {}


def _get_nc(scale: float) -> bass.Bass:
    if scale not in _CACHE:
        _CACHE[scale] = build_nc(scale)
    return _CACHE[scale]


def run(queries, keys, values, d_k, trace=False, trace_kwargs=None):
    scale = float(1.0 / np.sqrt(np.float32(d_k)))
    nc = _get_nc(scale)
    in_maps = _shard_inputs(queries, keys, values)
    res = bass_utils.run_bass_kernel_spmd(
        nc,
        in_maps,
        core_ids=list(range(N_CORES)),
        trace=trace,
        **(trace_kwargs or {}),
    )
    outT = np.stack([r["outT"] for r in res.results])  # [8, 8, D, S]
    out = outT.reshape(64, D, S).transpose(0, 2, 1)  # [64, S, D]
    out = np.ascontiguousarray(out).reshape(4, 16, S, D).astype(np.float32)
    return out, res


def kernel(queries, keys, values, d_k):
    out, _ = run(queries, keys, values, d_k, trace=False)
    return out
